# revision 10
# baseline (speedup 1.0000x reference)
"""Trainium2 Bass kernel for nn_AttentionBlock (B=16, C=512, H=W=32).

Math (identical to the verified baseline, reassociated):
  - GroupNorm(32, eps=1e-5), no affine -> hn [C, P], P = H*W flat (h*32+w).
  - The torch einsum `bHWHW,bcWH->bcWH` takes the softmax DIAGONAL, so all
    that survives of the attention is a per-position scale
        d[p=32h+w] = diagT[h, w],
        diagT[i,j] = 1024*exp(sc*S[33i,33j]) / sum_{h1,h2} exp(sc*S[32h1+i, 32h2+j])
    with S = hn^T (Wq Wk^T) hn, sc = C^-0.5 (the 1024 = position-count fold).
  - KEY reassociation: the diag scale commutes with the output projection:
        out = x + (1/65536) * D ∘ ((64*WvWn)^T hn)      D[c,p] = d[p]
    so Z = (64*WvWn)^T hn runs BEFORE the softmax completes, and the
    post-softmax work is elementwise only (short tail).

Precision: big matmuls fp8e4 DoubleRow.  x travels as bf16 (scaled 2^16,
exact pow2) both directions; host undoes the scale in fp32.  The attention
correction is ~2e-4 of ||x||, so bf16 residual noise (~1e-3) dominates the
error and sits far below the 2e-2 gate.

Sharding: data-parallel over batch, 2 per core, no collectives.
"""

import math
import os
import sys

import numpy as np

for _p in ("/opt/trn_rl_repo", "/opt/pypackages"):
    if os.path.isdir(_p) and _p not in sys.path:
        sys.path.append(_p)

import concourse.bass as bass
import concourse.mybir as mybir
import concourse.tile as tile
from concourse.bass_utils import run_bass_kernel_spmd

B, C, H, W = 16, 512, 32, 32
NPOS = H * W            # 1024
NCORES = 8
BPC = B // NCORES       # batches per core
KT = 4                  # 512 channels = 4 k-tiles of 128
EPS = 1e-5
SC = float(C) ** -0.5
WSCALE = 64.0           # host pre-scale on G / WVN for fp8 range
EXP_SCALE = SC / WSCALE
LN1024 = math.log(1024.0)
OUT_SCALE = 1.0 / (WSCALE * 1024.0)
XSCALE = 65536.0        # host pre-scale on x (= 1/OUT_SCALE, exact pow2)
EPS_DEV = EPS * XSCALE * XSCALE
F32 = mybir.dt.float32
F32R = mybir.dt.float32r
BF16 = mybir.dt.bfloat16
FP8 = mybir.dt.float8e4
AF = mybir.ActivationFunctionType
ALU = mybir.AluOpType
AX = mybir.AxisListType
DR = mybir.MatmulPerfMode.DoubleRow

# aux constant-tensor (fp32) column layout
A_F16 = 0             # [128, 8]    F16[p, g] = (p // 16 == g) / 16
A_E16 = 8             # [8, 128]    E16[g, q] = (q // 16 == g)
A_ONES32 = 136        # [32, 128]   ones (diag broadcast matmul, K=32)
NAUXF = 264
# fp8 merged-const column layout (bytes)
Q_G = 0               # [128, 4*512] g rearranged (k p) n -> p (k n)
Q_WVN = 2048
Q_FIND = 4096         # [128, 2*32] f_ind pair
Q_R32H = 4160         # [32, 1024]  R32H[k, n] = (n // 32 == k)  (0/1, exact fp8)
NQ = 5184


def _r(ap):
    return ap.bitcast(F32R)


def _split_sync_waits(nc, maxw=1):
    """walrus embeds at most one sync-wait per instruction; move extra waits
    onto preceding same-queue NoOps (FIFO queues keep semantics)."""
    n = 0
    for fn in nc.m.functions:
        for blk in fn.blocks:
            out = []
            for inst in blk.instructions:
                si = inst.sync_info
                waits = list(si.on_wait) if (si is not None and si.on_wait) else []
                if len(waits) > maxw:
                    keep = waits[-maxw:]
                    extra = waits[:-maxw]
                    for i in range(0, len(extra), maxw):
                        nop = mybir.InstNoOp(name=f"wsplit-{n}")
                        n += 1
                        nop.engine = inst.engine
                        nop.sync_info = mybir.SyncInfo(
                            on_wait=extra[i:i + maxw], on_update=[]
                        )
                        out.append(nop)
                    si.on_wait = keep
                out.append(inst)
            blk.instructions = out
    return n


def _build_nc():
    nc = bass.Bass()
    x_ext = nc.declare_dram_parameter("x", [BPC, C, NPOS], BF16, isOutput=False)
    aux_ext = nc.declare_dram_parameter("aux", [128, NAUXF], F32, isOutput=False)
    fq_ext = nc.declare_dram_parameter("fq", [128, NQ], FP8, isOutput=False)
    out_ext = nc.declare_dram_parameter("out", [BPC, C, NPOS], BF16, isOutput=True)

    with tile.TileContext(nc) as tc:
        from contextlib import ExitStack

        with ExitStack() as ctx:
            wpool = ctx.enter_context(tc.tile_pool(name="wpool", bufs=1))
            xpool = ctx.enter_context(tc.tile_pool(name="xpool", bufs=2))
            hnpool = ctx.enter_context(tc.tile_pool(name="hnpool", bufs=2))
            hhpool = ctx.enter_context(tc.tile_pool(name="hhpool", bufs=2))
            zpool = ctx.enter_context(tc.tile_pool(name="zpool", bufs=2))
            dpool = ctx.enter_context(tc.tile_pool(name="dpool", bufs=2))
            epool = ctx.enter_context(tc.tile_pool(name="epool", bufs=4))
            opool = ctx.enter_context(tc.tile_pool(name="opool", bufs=2))
            spool = ctx.enter_context(tc.tile_pool(name="spool", bufs=2))
            ps_s = ctx.enter_context(tc.tile_pool(name="ps_s", bufs=3, space="PSUM"))
            ps_z = ctx.enter_context(tc.tile_pool(name="ps_z", bufs=2, space="PSUM"))
            ps_r = ctx.enter_context(tc.tile_pool(name="ps_r", bufs=1, space="PSUM"))
            ps_m = ctx.enter_context(tc.tile_pool(name="ps_m", bufs=1, space="PSUM"))

            fq_sb = wpool.tile([128, NQ], FP8, tag="fq_sb", name="fq_sb")
            aux_sb = wpool.tile([128, NAUXF], F32R, tag="aux_sb", name="aux_sb")
            warm_sb = wpool.tile([128, 512], F32, tag="warm_sb", name="warm_sb")
            eps_sb = wpool.tile([128, 1], F32, tag="eps_sb", name="eps_sb")
            ln1024_sb = wpool.tile([128, 1], F32, tag="ln1024_sb", name="ln1024_sb")

            g_sb = fq_sb[:, Q_G:Q_G + 2048].rearrange("p (k n) -> p k n", k=KT)
            wvn_sb = fq_sb[:, Q_WVN:Q_WVN + 2048].rearrange("p (k n) -> p k n", k=KT)
            auxq_sb = fq_sb[:, Q_FIND:Q_FIND + 64].rearrange("p (a b) -> p a b", a=2)
            f16 = aux_sb[:, A_F16:A_F16 + 8]
            e16 = aux_sb[0:8, A_E16:A_E16 + 128]
            ones32 = aux_sb[0:32, A_ONES32:A_ONES32 + 128]
            r32h = fq_sb[0:32, Q_R32H:Q_R32H + NPOS]

            # single shared PSUM scratch: warmup, group-stat mms, numer
            ps_misc = ps_m.tile([128, 512], F32, tag="m", name="ps_misc")
            # psR (reused b0->b1 via subtile deps; matmul out must sit at
            # partition 0 / bank base, so no row-split halves)
            psr_all = ps_r.tile([32, NPOS], F32, tag="psr", name="psr_all")

            st = [dict() for _ in range(BPC)]

            # ps_misc holds only warmup/filler outs (never read), at the
            # bank base -- matmul outs must be bank-aligned
            def warmup(n):
                nc.vector.memset(warm_sb, 0.0)
                nc.vector.memset(eps_sb, EPS_DEV)
                nc.vector.memset(ln1024_sb, LN1024)
                # dense N=416 stream flips the HAM clock gate to 2.4GHz
                for _ in range(n):
                    nc.tensor.matmul(ps_misc, _r(warm_sb[:, 0:128]),
                                     _r(warm_sb),
                                     start=True, stop=True,
                                     skip_group_check=True)

            def filler_bf16(n, rhs_ap):
                """Keep-warm matmuls gated on real (bf16) data (bridge DMA/DVE
                waits so the HAM clock gate never re-throttles the PE)."""
                for _ in range(n):
                    nc.tensor.matmul(ps_misc,
                                     warm_sb.bitcast(BF16)[:, 0:128],
                                     rhs_ap,
                                     start=True, stop=True,
                                     skip_group_check=True)

            def filler_ep(n, ep):
                for _ in range(n):
                    nc.tensor.matmul(ps_misc[0:32, 0:256], auxq_sb,
                                     ep[:, :, 0:256],
                                     start=True, stop=True, perf_mode=DR,
                                     skip_group_check=True)

            def load_input_dmas():
                for b in range(BPC):
                    st[b]["x"] = [
                        xpool.tile([128, 2, NPOS], BF16, tag=f"x_sb{h}",
                                   name=f"x_sb{h}")
                        for h in range(2)
                    ]
                # ring ACT: fp8/f32 consts; g + aux first (gate stats/hh0)
                nc.scalar.dma_start(out=fq_sb[:, 0:2048], in_=fq_ext[:, 0:2048])
                nc.scalar.dma_start(out=aux_sb, in_=aux_ext[:, :].bitcast(F32R))
                nc.scalar.dma_start(out=fq_sb[:, 4096:NQ], in_=fq_ext[:, 4096:NQ])
                nc.scalar.dma_start(out=fq_sb[:, 2048:4096],
                                    in_=fq_ext[:, 2048:4096])
                # ring SP: all of x, batch 0 first (it gates the whole ramp)
                for b in range(BPC):
                    xv = x_ext[b].rearrange("(h p) n -> p h n", p=128)
                    nc.sync.dma_start(out=st[b]["x"][0], in_=xv[:, 0:2])
                    nc.sync.dma_start(out=st[b]["x"][1], in_=xv[:, 2:4])

            def xkt(b, kt):
                return st[b]["x"][kt // 2][:, kt % 2]

            def stats_pair(b, h, cast_engs):
                """GroupNorm stats + fp8 hn cast for one x half (kts 2h,2h+1).
                cast_engs: engines for the two hn casts ('act' or 'dve')."""
                s = st[b]
                if h == 0:
                    s["hn"] = hnpool.tile([128, KT, NPOS], FP8, tag="hn_sb",
                                          name="hn_sb")
                hn_sb = s["hn"]
                stats = spool.tile([128, 2, 2, 6], F32, tag=f"stats{h}",
                                   name=f"stats{h}")
                for j in range(2):
                    for sub in range(2):
                        nc.vector.bn_stats(
                            out=stats[:, j, sub, :],
                            in_=xkt(b, 2 * h + j)[:, sub * 512:(sub + 1) * 512],
                        )
                mv = spool.tile([128, 2, 2], F32, tag=f"mv{h}", name=f"mv{h}")
                for j in range(2):
                    nc.vector.bn_aggr(out=mv[:, j, :], in_=stats[:, j, :, :])
                rhs4 = spool.tile([128, 4], F32R, tag=f"rhs4_{h}",
                                  name=f"rhs4_{h}")
                nc.vector.tensor_copy(out=rhs4[:, 0:2], in_=mv[:, :, 0])
                nc.vector.tensor_tensor(
                    out=rhs4[:, 2:4], in0=mv[:, :, 0], in1=mv[:, :, 0], op=ALU.mult
                )
                nc.vector.tensor_tensor(
                    out=rhs4[:, 2:4], in0=rhs4[:, 2:4].bitcast(F32), in1=mv[:, :, 1],
                    op=ALU.add,
                )
                gt = ps_s.tile([128, 512], F32, tag="s", name="gst_slot")
                gst_ps = gt[0:8, 0:4]
                nc.tensor.matmul(gst_ps, _r(f16), _r(rhs4), start=True, stop=True,
                                 skip_group_check=True)
                # mu_inv: cols 0:2 = -mu_g, cols 2:4 = invsigma_g (per kt)
                mu_inv = spool.tile([8, 4], F32R, tag=f"mu_inv{h}",
                                    name=f"mu_inv{h}")
                nc.scalar.mul(out=mu_inv[:, 0:2], in_=gst_ps[:, 0:2], mul=-1.0)
                m2 = spool.tile([8, 2], F32, tag=f"m2_{h}", name=f"m2_{h}")
                nc.scalar.square(out=m2, in_=gst_ps[:, 0:2])
                var2 = spool.tile([8, 2], F32, tag=f"var2_{h}", name=f"var2_{h}")
                nc.vector.tensor_tensor(
                    out=var2, in0=gst_ps[:, 2:4], in1=m2, op=ALU.subtract
                )
                lnv = spool.tile([8, 2], F32, tag=f"lnv{h}", name=f"lnv{h}")
                nc.scalar.activation(out=lnv, in_=var2, func=AF.Ln,
                                     bias=eps_sb[0:8, :])
                nc.scalar.activation(out=mu_inv[:, 2:4], in_=lnv, func=AF.Exp,
                                     scale=-0.5)
                pt = ps_s.tile([128, 512], F32, tag="s", name="perch_slot")
                perch_ps = pt[:, 0:4]
                nc.tensor.matmul(perch_ps, _r(e16), _r(mu_inv), start=True,
                                 stop=True, skip_group_check=True)
                perch = spool.tile([128, 4], F32, tag=f"perch{h}",
                                   name=f"perch{h}")
                nc.vector.tensor_copy(out=perch, in_=perch_ps)
                negms = spool.tile([128, 2], F32, tag=f"negms{h}",
                                   name=f"negms{h}")
                nc.vector.tensor_tensor(
                    out=negms, in0=perch[:, 0:2], in1=perch[:, 2:4], op=ALU.mult
                )
                for j in range(2):
                    kt = 2 * h + j
                    if cast_engs[j] == "dve":
                        nc.vector.tensor_scalar(
                            out=hn_sb[:, kt],
                            in0=xkt(b, kt),
                            scalar1=perch[:, 2 + j:3 + j],
                            scalar2=negms[:, j:j + 1],
                            op0=ALU.mult,
                            op1=ALU.add,
                        )
                    else:
                        nc.scalar.activation(
                            out=hn_sb[:, kt],
                            in_=xkt(b, kt),
                            func=AF.Identity,
                            scale=perch[:, 2 + j:3 + j],
                            bias=negms[:, j:j + 1],
                        )

            def proj_half(b, mt, nh, w_sb, dst_key, dst_pool, dst_dt, ceng):
                """One [128,512] half of a weight^T @ hn projection (fp8 DR),
                copied/cast to SBUF on engine `ceng`."""
                s = st[b]
                hn_sb = s["hn"]
                if mt == 0 and nh == 0:
                    s[dst_key] = dst_pool.tile([128, KT, NPOS], dst_dt,
                                               tag=dst_key, name=dst_key)
                dst = s[dst_key]
                sl = slice(nh * 512, (nh + 1) * 512)
                ps = ps_z.tile([128, 512], F32, tag="z", name="proj_ps")
                for g in range(2):
                    nc.tensor.matmul(
                        ps,
                        w_sb[:, 2 * g:2 * g + 2, mt * 128:(mt + 1) * 128],
                        hn_sb[:, 2 * g:2 * g + 2, sl],
                        start=(g == 0),
                        stop=(g == 1),
                        perf_mode=DR,
                    )
                if ceng == "act":
                    nc.scalar.copy(out=dst[:, mt, sl], in_=ps)
                elif ceng == "dve":
                    nc.vector.tensor_copy(out=dst[:, mt, sl], in_=ps)
                else:
                    nc.gpsimd.tensor_copy(out=dst[:, mt, sl], in_=ps)

            def numer(b):
                """numT = 1024*exp(sc*S[33i,33j]) via strided fp8 matmul."""
                s = st[b]
                hn_sb, hh_sb = s["hn"], s["hh"]
                nt_slot = ps_s.tile([128, 512], F32, tag="s", name="nps_slot")
                nps = nt_slot[0:32, 0:32]
                for kt in range(KT):
                    nc.tensor.matmul(
                        nps,
                        hh_sb[:, kt, 0:NPOS:33],
                        hn_sb[:, kt, 0:NPOS:33],
                        start=(kt == 0),
                        stop=(kt == KT - 1),
                        skip_group_check=True,
                    )
                s["numT"] = numT = spool.tile([32, 32], F32, tag="numT", name="numT")
                nc.scalar.activation(out=numT, in_=nps, func=AF.Exp,
                                     scale=EXP_SCALE, bias=ln1024_sb[0:32, :])

            def s_phase(b, inject):
                """S tiles ([128,512] halves) -> exp(fp8) -> psR row-reduction.
                inject: {(nt, mh): [fns]} emitted after that chunk."""
                s = st[b]
                hn_sb, hh_sb = s["hn"], s["hh"]
                pairs = []
                e_pair = None
                for nt in range(8):
                    if nt % 2 == 0:
                        e_pair = epool.tile([128, 2, NPOS], FP8, tag="e_pair",
                                            name="e_pair")
                        pairs.append(e_pair)
                    for mh in range(2):
                        sl = slice(mh * 512, (mh + 1) * 512)
                        ps = ps_s.tile([128, 512], F32, tag="s", name="s_ps")
                        for g in range(2):
                            nc.tensor.matmul(
                                ps,
                                hh_sb[:, 2 * g:2 * g + 2, nt * 128:(nt + 1) * 128],
                                hn_sb[:, 2 * g:2 * g + 2, sl],
                                start=(g == 0),
                                stop=(g == 1),
                                perf_mode=DR,
                            )
                        nc.scalar.activation(out=e_pair[:, nt % 2, sl], in_=ps,
                                             func=AF.Exp, scale=EXP_SCALE)
                        if nt % 2 == 1:
                            pi = nt // 2
                            nc.tensor.matmul(
                                psr_all[:, sl],
                                auxq_sb,
                                pairs[pi][:, :, sl],
                                start=(pi == 0),
                                stop=(pi == 3),
                                perf_mode=DR,
                                skip_group_check=True,
                            )
                        for fn in inject.get((nt, mh), []):
                            fn()
                s["pairs"] = pairs

            def diag_chain(b):
                s = st[b]
                den = spool.tile([32, 32], F32, tag="den", name="den")
                nc.vector.tensor_reduce(
                    out=den,
                    in_=psr_all.rearrange("p (a b) -> p b a", a=32),
                    axis=AX.X,
                    op=ALU.add,
                )
                rden = spool.tile([32, 32], F32, tag="rden", name="rden")
                nc.vector.reciprocal(out=rden, in_=den)
                diagT = spool.tile([32, 32], F32, tag="diagT", name="diagT")
                nc.vector.tensor_tensor(out=diagT, in0=s["numT"], in1=rden,
                                        op=ALU.mult)
                s["diagT"] = diagT

            def d_bcast_half(b, nh):
                """D[c, n] = diagT[n//32, n%32] broadcast: mask-multiply on the
                DVE (stride-0 broadcast read) + K=32 ones matmul, then a bf16
                copy to SBUF so the PSUM slot frees immediately."""
                s = st[b]
                diagT = s["diagT"]
                sl = slice(nh * 512, (nh + 1) * 512)
                masked = spool.tile([32, 512], F32R, tag=f"msk{nh}",
                                    name=f"msk{nh}")
                nc.vector.tensor_tensor(
                    out=masked.rearrange("p (a b) -> p a b", a=16),
                    in0=r32h[:, sl].rearrange("p (a b) -> p a b", a=16),
                    in1=diagT.unsqueeze(1).broadcast_to([32, 16, 32]),
                    op=ALU.mult,
                )
                ps_d = ps_z.tile([128, 512], F32, tag="z", name="ps_d")
                nc.tensor.matmul(ps_d, _r(ones32), masked, start=True, stop=True)
                if nh == 0:
                    s["D"] = dpool.tile([128, 2, 512], BF16, tag="d_sb",
                                        name="d_sb")
                nc.scalar.copy(out=s["D"][:, nh], in_=ps_d)

            def mult_add(b, mt, eng_name):
                """o2[mt] = Z[mt] * D + 65536*x[mt]  (bf16, 2 passes, all
                SBUF operands so GpSimd is legal)."""
                s = st[b]
                eng = {"dve": nc.vector, "gp": nc.gpsimd}[eng_name]
                if mt % 2 == 0:
                    s["o2"] = opool.tile([128, 2, NPOS], BF16, tag="o_sb",
                                         name="o_sb")
                o2 = s["o2"]
                ov = o2[:, mt % 2].rearrange("p (a b) -> p a b", a=2)
                eng.tensor_tensor(
                    out=ov,
                    in0=s["z"][:, mt].rearrange("p (a b) -> p a b", a=2),
                    in1=s["D"],
                    op=ALU.mult,
                )
                eng.tensor_tensor(out=o2[:, mt % 2], in0=o2[:, mt % 2],
                                  in1=xkt(b, mt), op=ALU.add)

            def out_dma(b, pair, ring):
                s = st[b]
                ov = out_ext[b].rearrange("(k p) n -> p k n", p=128)
                ring.dma_start(out=ov[:, 2 * pair:2 * pair + 2, :], in_=s["o2"])

            def out_dma_single(b, mt, ring):
                s = st[b]
                ov = out_ext[b].rearrange("(k p) n -> p k n", p=128)
                ring.dma_start(out=ov[:, mt:mt + 1, :],
                               in_=s["o2"][:, mt % 2:mt % 2 + 1, :])

            # ---- pipelined emission over the two batches ----
            warmup(int(os.environ.get("TRN_WARM_N", "10")))
            load_input_dmas()
            stats_pair(0, 0, ("act", "dve"))
            filler_bf16(3, xkt(0, 0)[:, 0:512])
            stats_pair(0, 1, ("act", "dve"))
            filler_bf16(3, xkt(0, 2)[:, 0:512])
            # hh0: casts alternate ACT/DVE (ACT is free pre-S0)
            for mt in range(KT):
                for nh in range(2):
                    proj_half(0, mt, nh, g_sb, "hh", hhpool, FP8,
                              "act" if (2 * mt + nh) % 2 == 0 else "dve")
            numer(0)
            # z0: copies on GP/ACT (DVE will pick up b1 stats mid-S0)
            for mt in range(KT):
                for nh in range(2):
                    proj_half(0, mt, nh, wvn_sb, "z", zpool, BF16,
                              "dve" if (2 * mt + nh) % 2 == 0 else "act")
            # batch-1 front absorbs into batch-0's S phase.  All b1 elementwise
            # lands on DVE/GP so ACT keeps pace with the exp stream.
            s_phase(0, inject={
                (0, 1): [lambda: stats_pair(1, 0, ("dve", "dve"))],
                (1, 1): [lambda: stats_pair(1, 1, ("dve", "dve"))],
                (3, 1): [lambda: proj_half(1, 0, 0, g_sb, "hh", hhpool, FP8, "dve"),
                         lambda: proj_half(1, 0, 1, g_sb, "hh", hhpool, FP8, "dve")],
                (4, 1): [lambda: proj_half(1, 1, 0, g_sb, "hh", hhpool, FP8, "dve"),
                         lambda: proj_half(1, 1, 1, g_sb, "hh", hhpool, FP8, "dve")],
                (5, 1): [lambda: proj_half(1, 2, 0, g_sb, "hh", hhpool, FP8, "dve"),
                         lambda: proj_half(1, 2, 1, g_sb, "hh", hhpool, FP8, "dve")],
                (6, 1): [lambda: proj_half(1, 3, 0, g_sb, "hh", hhpool, FP8, "dve"),
                         lambda: proj_half(1, 3, 1, g_sb, "hh", hhpool, FP8, "dve")],
                (7, 1): [lambda: numer(1)],
            })
            diag_chain(0)
            d_bcast_half(0, 0)
            d_bcast_half(0, 1)
            # z1 interleaved with batch-0's elementwise tail work
            for mt in range(KT):
                for nh in range(2):
                    proj_half(1, mt, nh, wvn_sb, "z", zpool, BF16,
                              "dve" if nh == 0 else "act")
                mult_add(0, mt, "dve" if mt == 0 else "gp")
                if mt == 1:
                    out_dma(0, 0, nc.sync)
            out_dma(0, 1, nc.sync)
            s_phase(1, inject={})
            filler_ep(6, st[1]["pairs"][3])       # bridge diag1 chain, stay warm
            diag_chain(1)
            d_bcast_half(1, 0)
            filler_ep(4, st[1]["pairs"][3])
            d_bcast_half(1, 1)
            mult_add(1, 0, "dve")
            mult_add(1, 1, "gp")
            out_dma(1, 0, nc.sync)
            mult_add(1, 2, "dve")
            out_dma_single(1, 2, nc.sync)
            mult_add(1, 3, "gp")
            out_dma_single(1, 3, nc.scalar)
    if os.environ.get("TRN_NO_WAITSPLIT") != "1":
        _split_sync_waits(nc, maxw=1)
    return nc


def _make_aux():
    aux = np.zeros((128, NAUXF), np.float32)
    p = np.arange(128)
    aux[p, A_F16 + (p // 16) % 8] = 1.0 / 16.0
    for g in range(8):
        for q in range(128):
            if q // 16 == g:
                aux[g, A_E16 + q] = 1.0
    aux[0:32, A_ONES32:A_ONES32 + 128] = 1.0
    return aux


def _make_fq(G, WVN, FP8NP):
    """Merged fp8 consts: g / wvn rearranged (k p) n -> p (k n), f_ind pair."""
    fq = np.zeros((128, NQ), FP8NP)
    gr = G.reshape(KT, 128, C).transpose(1, 0, 2).reshape(128, KT * C)
    wr = WVN.reshape(KT, 128, C).transpose(1, 0, 2).reshape(128, KT * C)
    fq[:, Q_G:Q_G + 2048] = gr
    fq[:, Q_WVN:Q_WVN + 2048] = wr
    p = np.arange(128)
    fq[p, Q_FIND + p % 32] = 1.0
    fq[p, Q_FIND + 32 + p % 32] = 1.0
    n = np.arange(NPOS)
    for k in range(32):
        fq[k, Q_R32H:Q_R32H + NPOS] = (n // 32 == k).astype(np.float32)
    return fq


def _reference_numpy(x, Wq, bq, Wk, bk, Wv, bv, Wn, bn):
    """Exact (slow) numpy fallback, only used if biases are nonzero."""
    Bn_, C_, H_, W_ = x.shape
    xg = x.reshape(Bn_, 32, -1).astype(np.float64)
    mu = xg.mean(-1, keepdims=True)
    var = xg.var(-1, keepdims=True)
    h = ((xg - mu) / np.sqrt(var + EPS)).reshape(Bn_, C_, H_, W_).astype(np.float32)
    bqv = bq.reshape(1, C_, 1, 1)
    bkv = bk.reshape(1, C_, 1, 1)
    bvv = bv.reshape(1, C_, 1, 1)
    bnv = bn.reshape(1, C_, 1, 1)

    def nin(t, Wm, bb):
        return np.einsum("bchw,co->bowh", t, Wm, optimize=True) + bb

    q = nin(h, Wq, bqv)
    k = nin(h, Wk, bkv)
    v = nin(h, Wv, bvv)
    out = np.empty_like(x)
    sc = C_ ** -0.5
    for bi in range(Bn_):
        Q = q[bi].transpose(2, 1, 0).reshape(-1, C_)
        K = k[bi].transpose(2, 1, 0).reshape(-1, C_)
        S = (Q @ K.T) * sc
        S5 = S.reshape(H_, W_, H_, W_).transpose(1, 3, 0, 2)
        Sm = S5.reshape(W_, W_, -1)
        Sm = Sm - Sm.max(-1, keepdims=True)
        E = np.exp(Sm)
        SMX = (E / E.sum(-1, keepdims=True)).reshape(W_, W_, H_, H_)
        ii = np.arange(H_)
        jj = np.arange(W_)
        diag = SMX[ii[:, None], jj[None, :], ii[:, None], jj[None, :]]
        h2v = v[bi] * np.swapaxes(diag, 0, 1)[None]
        out[bi] = np.einsum("cwh,co->ohw", h2v, Wn, optimize=True) + bnv[0]
    return (x + out).astype(np.float32)


_NC_CACHE = None


def kernel(**inputs):
    x = np.ascontiguousarray(np.asarray(inputs["x"], dtype=np.float32))
    Wq = np.asarray(inputs["Wq"], dtype=np.float32)
    Wk = np.asarray(inputs["Wk"], dtype=np.float32)
    Wv = np.asarray(inputs["Wv"], dtype=np.float32)
    Wn = np.asarray(inputs["Wn"], dtype=np.float32)
    bq = np.asarray(inputs["bq"], dtype=np.float32)
    bk = np.asarray(inputs["bk"], dtype=np.float32)
    bv = np.asarray(inputs["bv"], dtype=np.float32)
    bn = np.asarray(inputs["bn"], dtype=np.float32)

    if any(np.any(bb != 0) for bb in (bq, bk, bv, bn)):
        return _reference_numpy(x, Wq, bq, Wk, bk, Wv, bv, Wn, bn)

    import ml_dtypes

    FP8NP = ml_dtypes.float8_e4m3
    BF16NP = ml_dtypes.bfloat16
    G = np.clip(Wq @ Wk.T * WSCALE, -240, 240).astype(FP8NP)
    WVN = np.clip(Wv @ Wn * WSCALE, -240, 240).astype(FP8NP)
    aux = _make_aux()
    fq = _make_fq(G, WVN, FP8NP)

    global _NC_CACHE
    if _NC_CACHE is None:
        _NC_CACHE = _build_nc()
    nc = _NC_CACHE

    # exact pow2 scale, undone on host after the bf16 round-trip
    xf = (x * XSCALE).reshape(B, C, NPOS).astype(BF16NP)
    in_maps = [
        {
            "x": np.ascontiguousarray(xf[c * BPC:(c + 1) * BPC]),
            "aux": aux,
            "fq": fq,
        }
        for c in range(NCORES)
    ]
    trace = bool(int(os.environ.get("TRN_KERNEL_TRACE", "0")))
    res = run_bass_kernel_spmd(nc, in_maps, core_ids=list(range(NCORES)), trace=trace)
    if trace:
        kernel.last_exec_time_ns = res.exec_time_ns
        kernel.last_results = res
    out = np.empty((B, C, NPOS), np.float32)
    for c in range(NCORES):
        # device emits bf16 65536*(x + correction); undo the exact pow2 scale
        out[c * BPC:(c + 1) * BPC] = res.results[c]["out"].astype(np.float32)
    out *= OUT_SCALE
    return out.reshape(B, C, H, W)
